# revision 34
# baseline (speedup 1.0000x reference)
"""CortexIIBlock TRN2 Bass kernel v4 — fp8 DoubleRow GEMMs, DVE-lean ops.

8-core data-parallel over (batch, seq-half): each core owns 2048 sequence
positions. All big GEMMs run as fp8(e4m3) DoubleRow matmuls with an
error-compensated 3-pair split: for A ~= Ah+Al (hi + residual, same scale)
and W ~= Wh+Wl, accumulate Wh.Ah + Wh.Al + Wl.Ah in fp32 PSUM. Each
DoubleRow instruction carries two (weights, ifmap) k-pairs at 0.5
cycles/row, so the 3-pair scheme costs 0.75 cycles per 128-K tile per
output column vs 1.0 for bf16 -- a 1.33x PE speedup at ~2e-3 rel error.
Activations are pre-scaled (x16 / x8, folded into existing ops) so fp8
operands sit in e4m3's normal range; descales fold into the post-PSUM
activation/STT scales. Depthwise convs + softmax mixing stay bf16 on
DVE/GpSimd. Causal conv history (16 cols) is precomputed on the host.
"""
import numpy as np

D = 1024
DFF = 4096
B = 4
S = 4096
H = 16           # conv history cols (lookback <= 6, padded to 16)
R = 2048         # payload cols per shard
NCT = D // 128   # 8 channel tiles
NB = 4           # payload blocks
BN = 512
EPS = 1e-6

SA = 16.0        # activation scale for h, z(fused*gate), h2
SAP = 8.0        # activation scale for p = silu(g)*u
SW_UP = 1024.0
SW_DN = 1024.0
SW_G = 1024.0
SW_U = 1024.0
SW_O = 2048.0
SW_SG = 2048.0

_CACHE = {}


def _build():
    import concourse.bacc as bacc
    import concourse.mybir as mybir
    import concourse.tile as tile
    from concourse.bass import AP as BassAP

    # Steer the act-table pass to the combined ln+exp table: drop Exp/Ln
    # from the earlier-indexed single-function tables so both rmsnorm ops
    # resolve to natural_log_exp_and_others (canonical indices preserved;
    # the real hardware tables still contain the dropped entries).
    if not getattr(bacc, "_act_tables_patched", False):
        _orig_gat = bacc.get_activation_tables

        def _gat(arch):
            tabs = {k: set(v) for k, v in _orig_gat(arch).items()}
            AFt = mybir.ActivationFunctionType
            if "natural_log_exp_and_others" in tabs:
                tabs.get("exp_and_others", set()).discard(AFt.Exp)
                tabs.get("natural_log", set()).discard(AFt.Ln)
            return tabs

        bacc.get_activation_tables = _gat
        bacc._act_tables_patched = True

    F32 = mybir.dt.float32
    BF16 = mybir.dt.bfloat16
    F8 = mybir.dt.float8e4
    AF = mybir.ActivationFunctionType
    MUL = mybir.AluOpType.mult
    ADD = mybir.AluOpType.add
    SUB = mybir.AluOpType.subtract
    DR = mybir.MatmulPerfMode.DoubleRow

    nc = bacc.Bacc(None, target_bir_lowering=False)
    _lp = nc.allow_low_precision(reason="fp8 3-pair GEMMs within tolerance")
    _lp.__enter__()

    xT_d = nc.dram_tensor("xT", [128, NCT, R], BF16, kind="ExternalInput")
    vh_d = nc.dram_tensor("vh", [128, NCT, H], BF16, kind="ExternalInput")
    up8_d = nc.dram_tensor("up8", [16, 128, 4, 2, 2, 128], F8, kind="ExternalInput")
    dn8_d = nc.dram_tensor("dn8", [8, 128, 4, 2, 2, 128], F8, kind="ExternalInput")
    wgu8_d = nc.dram_tensor("wgu8", [32, 128, 2, 4, 2, 2, 128], F8, kind="ExternalInput")
    wo8_d = nc.dram_tensor("wo8", [8, 4, 128, 4, 2, 2, 128], F8, kind="ExternalInput")
    sg8_d = nc.dram_tensor("sg8", [128, 4, 2, 2, 16], F8, kind="ExternalInput")
    taps_d = nc.dram_tensor("taps_p", [128, NCT, 15], F32, kind="ExternalInput")
    ones128_d = nc.dram_tensor("ones128", [128, 1], BF16, kind="ExternalInput")
    one1_d = nc.dram_tensor("one1", [1, 128], BF16, kind="ExternalInput")
    sa1_d = nc.dram_tensor("sa1", [1, 128], BF16, kind="ExternalInput")
    yT_d = nc.dram_tensor("yT", [128, NCT, R], F32, kind="ExternalOutput")

    from contextlib import ExitStack
    with tile.TileContext(nc) as tc:
        with ExitStack() as stack:
            ep = stack.enter_context
            cpool = ep(tc.tile_pool(name="const", bufs=1))
            xp = ep(tc.tile_pool(name="xp", bufs=3))
            scr = ep(tc.tile_pool(name="scr", bufs=3))       # h_s / z_s / h2_s
            a8p = ep(tc.tile_pool(name="a8p", bufs=4))       # h8/z8/h28 rotate
            vp = ep(tc.tile_pool(name="vp", bufs=2))
            gp = ep(tc.tile_pool(name="gp", bufs=2))
            x2p = ep(tc.tile_pool(name="x2p", bufs=2))
            cvp = ep(tc.tile_pool(name="cvp", bufs=2))
            p8p = ep(tc.tile_pool(name="p8p", bufs=1))       # 16 pair tags
            tgp = ep(tc.tile_pool(name="tg", bufs=2))
            yp = ep(tc.tile_pool(name="yp", bufs=2))
            sqp = ep(tc.tile_pool(name="sqq", bufs=2))
            smp = ep(tc.tile_pool(name="sm", bufs=2))
            sbp = ep(tc.tile_pool(name="sb", bufs=2))
            wup = ep(tc.tile_pool(name="wup", bufs=4))       # up/down stream
            wgp = ep(tc.tile_pool(name="wgp", bufs=3))       # wg/wu stream
            wop = ep(tc.tile_pool(name="wop", bufs=4))       # wo halves
            psmm = ep(tc.tile_pool(name="psmm", bufs=5, space="PSUM"))
            psbc = ep(tc.tile_pool(name="psbc", bufs=1, space="PSUM"))
            psrd = ep(tc.tile_pool(name="psrd", bufs=2, space="PSUM"))

            # ---------------- constants ----------------
            ones128 = cpool.tile([128, 1], BF16, tag="c_ones", name="c_ones")
            nc.sync.dma_start(ones128[:], ones128_d[:])
            one1 = cpool.tile([1, 128], BF16, tag="c_one1", name="c_one1")
            nc.sync.dma_start(one1[:], one1_d[:])
            sa1 = cpool.tile([1, 128], BF16, tag="c_sa1", name="c_sa1")
            nc.sync.dma_start(sa1[:], sa1_d[:])
            eps_t = cpool.tile([1, 1], F32, tag="c_eps", name="c_eps")
            nc.vector.memset(eps_t[:], EPS)
            sg8_t = cpool.tile([128, 4, 2, 2, 16], F8, tag="c_sg", name="c_sg")
            nc.sync.dma_start(sg8_t[:], sg8_d[:])
            taps_t = cpool.tile([128, NCT, 15], F32, tag="c_taps", name="c_taps")
            nc.sync.dma_start(taps_t[:], taps_d[:])

            # ---------------- persistent per-block state ----------------
            xb = [None] * NB
            zs_t = [None] * NB
            h8 = [None] * NB
            z8 = [None] * NB
            h28 = [None] * NB
            val = [None] * NB
            gate = [None] * NB
            swb = [None] * NB
            x2 = [None] * NB
            p8 = [None] * NB

            def dr_gemm(ps, wt, a8, J, wsel=None, cs=0, n=BN):
                # 3-pair, Al-dependent instructions last (lo quant can lag)
                nn, tot = 0, 3 * J
                for (g, hh) in ((0, 0), (1, 0), (0, 1)):
                    for j in range(J):
                        wap = (wt[:, j, g] if wsel is None
                               else wt[:, wsel, j, g])
                        nc.tensor.matmul(
                            ps, wap, a8[:, hh, 2 * j:2 * j + 2, cs:cs + n],
                            start=(nn == 0), stop=(nn == tot - 1), perf_mode=DR)
                        nn += 1

            def rmsnorm_quant(src, a8_, tag, hold, cs=0, n=BN):
                # stats: sq (Act), partition-sum (PE), rstd (Act), bcast (PE+Act)
                ce = cs + n
                msum = psrd.tile([1, BN], F32, tag="msum", name=f"msum_{tag}{cs}")
                for c in range(NCT):
                    sq = sqp.tile([128, BN], BF16, tag="sq", name=f"sq_{tag}{cs}_{c}")
                    nc.scalar.activation(sq[:, :n], src[:, c, cs:ce], AF.Square)
                    nc.tensor.matmul(msum[:, :n], ones128[:], sq[:, :n],
                                     start=(c == 0), stop=(c == NCT - 1))
                sd = smp.tile([1, BN], F32, tag="sd", name=f"sd_{tag}{cs}", bufs=1)
                nc.scalar.activation(sd[:, :n], msum[:, :n], AF.Ln,
                                     bias=eps_t[:], scale=1.0 / D)
                rstd = smp.tile([1, BN], BF16, tag="rstd", name=f"rstd_{tag}{cs}")
                nc.scalar.activation(rstd[:, :n], sd[:, :n], AF.Exp, scale=-0.5)
                rsb_ps = psbc.tile([128, BN], F32, tag="pbc", name=f"rsbp_{tag}{cs}")
                nc.tensor.matmul(rsb_ps[:, :n], sa1[:], rstd[:, :n],
                                 start=True, stop=True)
                rsb = sbp.tile([128, BN], BF16, tag="rsb", name=f"rsb_{tag}{cs}", bufs=1)
                nc.scalar.copy(rsb[:, :n], rsb_ps[:, :n])
                if cs == 0:
                    hold[0] = scr.tile([128, NCT, BN], BF16, tag="scr",
                                       name=f"hs_{tag}")
                hs = hold[0]
                rap = rsb[:, 0:n]
                rbc = BassAP(rap.tensor, rap.offset,
                             [rap.ap[0], [0, NCT], rap.ap[1]])
                nc.vector.tensor_mul(hs[:, :, cs:ce], src[:, :, cs:ce], rbc)
                # quantize in halves so GEMM chains can start on half 0
                for hf in range(2):
                    c0, c1 = hf * 4, hf * 4 + 4
                    nc.scalar.activation(a8_[:, 0, c0:c1, cs:ce],
                                         hs[:, c0:c1, cs:ce], AF.Copy)
                for hf in range(2):
                    c0, c1 = hf * 4, hf * 4 + 4
                    nc.vector.scalar_tensor_tensor(
                        out=a8_[:, 1, c0:c1, cs:ce], in0=hs[:, c0:c1, cs:ce],
                        scalar=1.0, in1=a8_[:, 0, c0:c1, cs:ce], op0=MUL, op1=SUB)

            # ---------------- per-block phases ----------------
            fr_hs = [[None] for _ in range(NB)]
            ff_hs = [[None] for _ in range(NB)]

            def front(i, cs=0, n=BN):
                if cs == 0:
                    xb[i] = xp.tile([128, NCT, BN], BF16, tag="xb", name=f"xb{i}")
                    h8[i] = a8p.tile([128, 2, NCT, BN], F8, tag="a8", name=f"h8_{i}")
                for hf in range(2):
                    c0, c1 = hf * 4, hf * 4 + 4
                    nc.sync.dma_start(xb[i][:, c0:c1, cs:cs + n],
                                      xT_d[:, c0:c1, i * BN + cs:i * BN + cs + n])
                rmsnorm_quant(xb[i], h8[i], f"m{i}", fr_hs[i], cs, n)

            def sgup(i, cs=0, n=BN):
                a8_ = h8[i]
                ce = cs + n
                ej = []
                for j in range(3):
                    pj = psrd.tile([1, BN], F32, tag="msum", name=f"psg{i}_{j}{cs}")
                    nn = 0
                    for (g, hh) in ((0, 0), (1, 0), (0, 1)):
                        for jj in range(4):
                            nc.tensor.matmul(
                                pj[:, :n], sg8_t[:, jj, g, :, j:j + 1],
                                a8_[:, hh, 2 * jj:2 * jj + 2, cs:ce],
                                start=(nn == 0), stop=(nn == 11), perf_mode=DR)
                            nn += 1
                    e_ = smp.tile([1, BN], BF16, tag=f"e{j}", name=f"e{i}_{j}{cs}", bufs=1)
                    nc.scalar.activation(e_[:, :n], pj[:, :n], AF.Exp,
                                         scale=1.0 / (SA * SW_SG))
                    ej.append(e_)
                es = smp.tile([1, BN], BF16, tag="es", name=f"es{i}{cs}", bufs=1)
                nc.vector.tensor_add(es[:, :n], ej[0][:, :n], ej[1][:, :n])
                nc.vector.tensor_add(es[:, :n], es[:, :n], ej[2][:, :n])
                erec = smp.tile([1, BN], BF16, tag="erec", name=f"erec{i}{cs}", bufs=1)
                nc.vector.reciprocal(erec[:, :n], es[:, :n])
                if cs == 0:
                    swb[i] = [sbp.tile([128, BN], BF16, tag=f"swb{j}",
                                       name=f"swb{i}_{j}", bufs=1)
                              for j in range(3)]
                for j in range(3):
                    swj = smp.tile([1, BN], BF16, tag="swj", name=f"swj{i}_{j}{cs}", bufs=1)
                    nc.vector.tensor_mul(swj[:, :n], ej[j][:, :n], erec[:, :n])
                    pb_ = psbc.tile([128, BN], F32, tag="pbc", name=f"pswb{i}_{j}{cs}")
                    nc.tensor.matmul(pb_[:, :n], one1[:], swj[:, :n],
                                     start=True, stop=True)
                    nc.scalar.copy(swb[i][j][:, cs:ce], pb_[:, :n])

                # val half of up projection (m-tiles 8..15)
                if cs == 0:
                    val[i] = [vp.tile([128, H + BN], BF16, tag=f"val{c}",
                                      name=f"val{i}_{c}") for c in range(NCT)]
                for m in range(NCT):
                    wt = wup.tile([128, 4, 2, 2, 128], F8, tag="wup",
                                  name=f"wv{i}_{m}{cs}")
                    nc.sync.dma_start(wt[:], up8_d[8 + m])
                    pv = psmm.tile([128, BN], F32, tag="pmm", name=f"pval{i}_{m}{cs}")
                    dr_gemm(pv[:, :n], wt, a8_, 4, cs=cs, n=n)
                    # val scaled x SA: PSUM/(SA*SW_UP) * SA = PSUM/SW_UP
                    nc.scalar.activation(val[i][m][:, H + cs:H + ce], pv[:, :n],
                                         AF.Copy, scale=1.0 / SW_UP)
                    if cs == 0:
                        if i == 0:
                            nc.sync.dma_start(val[i][m][:, 0:H], vh_d[:, m, :])
                        else:
                            nc.vector.tensor_copy(val[i][m][:, 0:H],
                                                  val[i - 1][m][:, BN:BN + H])

                # gate half of up projection (m-tiles 0..7)
                if cs == 0:
                    gate[i] = gp.tile([128, NCT, BN], BF16, tag="gate", name=f"gate{i}")
                for m in range(NCT):
                    wt = wup.tile([128, 4, 2, 2, 128], F8, tag="wup",
                                  name=f"wg{i}_{m}{cs}")
                    nc.sync.dma_start(wt[:], up8_d[m])
                    pg = psmm.tile([128, BN], F32, tag="pmm", name=f"pgate{i}_{m}{cs}")
                    dr_gemm(pg[:, :n], wt, a8_, 4, cs=cs, n=n)
                    nc.scalar.activation(gate[i][:, m, cs:ce], pg[:, :n], AF.Sigmoid,
                                         scale=1.0 / (SA * SW_UP))

            def convmix(i, cs=0, n=BN, lowp_force=False):
                # conv taps as TS (4x mode) + TT adds (2x mode) on DVE;
                # softmax-weighted mix on GpSimd. Steady-state at low
                # priority (gap filler; deadline down(i)).
                ce = cs + n
                if cs == 0:
                    zs_t[i] = scr.tile([128, NCT, BN], BF16, tag="scr", name=f"zs{i}")
                    z8[i] = a8p.tile([128, 2, NCT, BN], F8, tag="a8", name=f"z8_{i}")
                z_ = zs_t[i]
                sw_ = swb[i]
                g_ = gate[i]
                lowp = None
                if i > 0 or lowp_force:
                    lowp = tc.high_priority(offset=-10_000_000)
                    lowp.__enter__()
                for c in range(NCT):
                    mix_eng = nc.gpsimd
                    v_ = val[i][c]
                    convs = []
                    for (nt, base) in ((3, 0), (5, 3), (7, 8)):
                        b = len(convs)
                        ct_ = cvp.tile([128, BN], BF16, tag=f"cv{b}",
                                       name=f"cv{i}_{c}_{b}{cs}")
                        nc.vector.tensor_scalar_mul(
                            ct_[:, :n], v_[:, H + cs:H + ce],
                            taps_t[:, c, base:base + 1])
                        for j in range(1, nt):
                            tm_ = cvp.tile([128, BN], BF16, tag="ctmp",
                                           name=f"ctmp{i}_{c}_{b}_{j}{cs}")
                            nc.vector.tensor_scalar_mul(
                                tm_[:, :n], v_[:, H + cs - j:H + ce - j],
                                taps_t[:, c, base + j:base + j + 1])
                            nc.vector.tensor_add(ct_[:, :n], ct_[:, :n], tm_[:, :n])
                        convs.append(ct_)
                    acc = cvp.tile([128, BN], BF16, tag="acc", name=f"acc{i}_{c}{cs}")
                    mix_eng.tensor_mul(acc[:, :n], convs[0][:, :n], sw_[0][:, cs:ce])
                    for j in (1, 2):
                        u_ = cvp.tile([128, BN], BF16, tag="mixu", name=f"mixu{i}_{c}{cs}")
                        mix_eng.tensor_mul(u_[:, :n], convs[j][:, :n], sw_[j][:, cs:ce])
                        mix_eng.tensor_add(acc[:, :n], acc[:, :n], u_[:, :n])
                    mix_eng.tensor_mul(z_[:, c, cs:ce], acc[:, :n], g_[:, c, cs:ce])
                if lowp is not None:
                    lowp.__exit__(None, None, None)
                # quantize z fully on GpSimd, in c-pair chunks
                z8_ = z8[i]
                for qq in range(4):
                    c0, c1 = qq * 2, qq * 2 + 2
                    nc.gpsimd.tensor_copy(z8_[:, 0, c0:c1, cs:ce], z_[:, c0:c1, cs:ce])
                    nc.gpsimd.tensor_sub(z8_[:, 1, c0:c1, cs:ce], z_[:, c0:c1, cs:ce],
                                         z8_[:, 0, c0:c1, cs:ce])

            def down(i, cs=0, n=BN):
                ce = cs + n
                if cs == 0:
                    x2[i] = x2p.tile([128, NCT, BN], BF16, tag="x2", name=f"x2_{i}")
                for m in range(NCT):
                    wt = wup.tile([128, 4, 2, 2, 128], F8, tag="wup",
                                  name=f"wd{i}_{m}{cs}")
                    nc.sync.dma_start(wt[:], dn8_d[m])
                    pm = psmm.tile([128, BN], F32, tag="pmm", name=f"pmix{i}_{m}{cs}")
                    dr_gemm(pm[:, :n], wt, z8[i], 4, cs=cs, n=n)
                    nc.vector.scalar_tensor_tensor(
                        out=x2[i][:, m, cs:ce], in0=pm[:, :n],
                        scalar=1.0 / (SA * SW_DN),
                        in1=xb[i][:, m, cs:ce], op0=MUL, op1=ADD)

            def ffnf(i, cs=0, n=BN):
                if cs == 0:
                    h28[i] = a8p.tile([128, 2, NCT, BN], F8, tag="a8", name=f"h28_{i}")
                rmsnorm_quant(x2[i], h28[i], f"f{i}", ff_hs[i], cs, n)

            def gup(i, cs=0, n=BN):
                ce = cs + n
                if cs == 0:
                    p8[i] = [p8p.tile([128, 2, BN], F8, tag=f"p8_{q}",
                                      name=f"p8_{i}_{q}") for q in range(16)]
                for m in range(32):
                    wt = wgp.tile([128, 2, 4, 2, 2, 128], F8, tag="wgu",
                                  name=f"wgu{i}_{m}{cs}")
                    nc.sync.dma_start(wt[:], wgu8_d[m])
                    pg = psmm.tile([128, BN], F32, tag="pmm", name=f"pg{i}_{m}{cs}")
                    dr_gemm(pg[:, :n], wt, h28[i], 4, wsel=0, cs=cs, n=n)
                    tg = tgp.tile([128, BN], BF16, tag="tg", name=f"tg{i}_{m}{cs}")
                    nc.scalar.activation(tg[:, :n], pg[:, :n], AF.Silu,
                                         scale=1.0 / (SA * SW_G))
                    pu = psmm.tile([128, BN], F32, tag="pmm", name=f"pu{i}_{m}{cs}")
                    dr_gemm(pu[:, :n], wt, h28[i], 4, wsel=1, cs=cs, n=n)
                    q, t = m // 2, m % 2
                    # p8 hi written directly from PSUM: (pu*s)*silu(g) -> fp8
                    nc.vector.scalar_tensor_tensor(
                        out=p8[i][q][:, t, cs:ce], in0=pu[:, :n],
                        scalar=SAP / (SA * SW_U), in1=tg[:, :n], op0=MUL, op1=MUL)

            def ffn_out(i):
                for m in range(NCT):
                    wq = []
                    for qq in range(4):
                        w_ = wop.tile([128, 4, 2, 2, 128], F8, tag="wo",
                                      name=f"wo{i}_{m}_{qq}")
                        nc.sync.dma_start(w_[:], wo8_d[m, qq])
                        wq.append(w_)
                    py = psmm.tile([128, BN], F32, tag="pmm", name=f"py{i}_{m}")
                    nn = 0
                    for q in range(16):
                        wt = wq[q // 4]
                        jj = q % 4
                        for g in (0, 1):  # 2-pair: (Woh, P), (Wol, P)
                            nc.tensor.matmul(
                                py[:], wt[:, jj, g], p8[i][q][:],
                                start=(nn == 0), stop=(nn == 31), perf_mode=DR)
                            nn += 1
                    yo = yp.tile([128, BN], F32, tag="yo", name=f"yo{i}_{m}")
                    nc.vector.scalar_tensor_tensor(
                        out=yo[:], in0=py[:], scalar=1.0 / (SAP * SW_O),
                        in1=x2[i][:, m, :], op0=MUL, op1=ADD)
                    nc.sync.dma_start(yT_d[:, m, i * BN:(i + 1) * BN], yo[:])

            # ---------------- schedule ----------------
            front(0)
            sgup(0)
            front(1)
            convmix(0)
            sgup(1)
            front(2)
            down(0)
            ffnf(0)
            convmix(1)
            for i in range(NB):
                gup(i)
                if i + 3 < NB:
                    front(i + 3)
                if i + 1 < NB:
                    # elevated: the next block's residual+norm chain gates
                    # gup(i+1); let it preempt silu/pt backlogs on Act/DVE
                    hp = tc.high_priority(offset=10_000_000)
                    hp.__enter__()
                    down(i + 1)
                    ffnf(i + 1)
                    hp.__exit__(None, None, None)
                ffn_out(i)
                if i + 2 < NB:
                    sgup(i + 2)
                    convmix(i + 2)

    if not nc.is_finalized():
        nc.finalize()
    return nc


def _host_prep(x, ln1_w, ln2_w, w_fine, w_medium, w_coarse, sg_w, up_w, down_w, wg, wu, wo):
    import ml_dtypes
    f = np.float32
    bf = ml_dtypes.bfloat16
    f8 = ml_dtypes.float8_e4m3

    def wsplit_pack(w, sw):
        # w [F, D] -> [F//128, 128, D//256, 2(hi/lo), 2(ktile), 128] fp8
        F_, D_ = w.shape
        ws = np.asarray(w, f) * sw
        hi = np.clip(ws, -240, 240).astype(f8)
        lo = (ws - hi.astype(f)).astype(f8)

        def pack(src):
            a = src.reshape(F_ // 128, 128, D_ // 256, 2, 128)  # m, col, j, t, part
            return a.transpose(0, 4, 2, 3, 1)                   # m, part, j, t, col

        return np.ascontiguousarray(np.stack([pack(hi), pack(lo)], axis=3))

    # fold the rmsnorm elementwise weights into the matmul weight columns
    ln1f = np.asarray(ln1_w, f)
    ln2f = np.asarray(ln2_w, f)
    up_l = np.asarray(up_w, f) * ln1f[None, :]
    sg_l = np.asarray(sg_w, f) * ln1f[None, :]
    wg_l = np.asarray(wg, f) * ln2f[None, :]
    wu_l = np.asarray(wu, f) * ln2f[None, :]

    up8 = wsplit_pack(up_l, SW_UP)          # [16, 128, 4, 2, 2, 128]
    dn8 = wsplit_pack(np.asarray(down_w, f), SW_DN)
    wgu8 = np.ascontiguousarray(np.stack(
        [wsplit_pack(wg_l, SW_G), wsplit_pack(wu_l, SW_U)], axis=2))
    wo8_flat = wsplit_pack(np.asarray(wo, f), SW_O)   # [8, 128, 16, 2, 2, 128]
    wo8 = np.ascontiguousarray(
        wo8_flat.reshape(8, 128, 4, 4, 2, 2, 128).transpose(0, 2, 1, 3, 4, 5, 6))

    sgs = sg_l * SW_SG                      # [3, 1024]
    sgh = np.clip(sgs, -240, 240).astype(f8)
    sgl = (sgs - sgh.astype(f)).astype(f8)

    def sg_pack(src):
        a = np.zeros((1024, 16), src.dtype)
        a[:, :3] = src.T
        a = a.reshape(4, 2, 128, 16)        # j, t, part, col
        return a.transpose(2, 0, 1, 3)      # part, j, t, col

    sg8 = np.ascontiguousarray(np.stack([sg_pack(sgh), sg_pack(sgl)], axis=2))

    taps = np.zeros((NCT, 128, 15), f)
    for (w_, nt, base) in ((w_fine, 3, 0), (w_medium, 5, 3), (w_coarse, 7, 8)):
        for j in range(nt):
            taps[:, :, base + j] = np.asarray(w_, f)[:, 0, nt - 1 - j].reshape(NCT, 128)
    taps_p = np.ascontiguousarray(taps.transpose(1, 0, 2))

    shared = dict(up8=up8, dn8=dn8, wgu8=wgu8, wo8=wo8, sg8=sg8,
                  taps_p=taps_p,
                  ones128=np.ones((128, 1), bf), one1=np.ones((1, 128), bf),
                  sa1=np.full((1, 128), SA, bf))

    xf = np.asarray(x, f)
    upv_l = up_l[D:2 * D]                   # ln-folded val half [D, D]
    in_maps = []
    for core in range(8):
        b, half = core // 2, core % 2
        pay = xf[b, half * R:(half + 1) * R]                   # [R, D]
        xTh = np.ascontiguousarray(
            pay.reshape(R, NCT, 128).transpose(2, 1, 0)).astype(bf)
        # host-computed conv history: val of the 16 tokens before this shard
        if half == 0:
            vh16 = np.zeros((H, D), f)
        else:
            hist = xf[b, R - H:R]                              # [H, D]
            ms = np.mean(hist * hist, axis=-1, keepdims=True)
            hh = hist / np.sqrt(ms + EPS)
            vh16 = hh @ upv_l.T                                # [H, D]
        vhT = np.ascontiguousarray(
            (vh16 * SA).reshape(H, NCT, 128).transpose(2, 1, 0)).astype(bf)
        in_maps.append({**shared, "xT": xTh, "vh": vhT})
    return in_maps


def kernel(**inputs):
    from concourse.bass_utils import run_bass_kernel_spmd
    if "nc" not in _CACHE:
        _CACHE["nc"] = _build()
    nc = _CACHE["nc"]
    in_maps = _host_prep(**{k: np.asarray(v) for k, v in inputs.items()})
    res = run_bass_kernel_spmd(nc, in_maps, core_ids=list(range(8)))
    out = np.empty((B, S, D), np.float32)
    for core in range(8):
        b, half = core // 2, core % 2
        yTh = res.results[core]["yT"]                 # [128, NCT, R]
        out[b, half * R:(half + 1) * R] = yTh.transpose(2, 1, 0).reshape(R, D)
    return out
